# revision 15
# baseline (speedup 1.0000x reference)
"""Trainium2 Bass kernel for a biquad lowpass filter over [256, 160000] audio.

Algorithm
---------
The reference is a Direct-Form-I biquad IIR (lowpass, cutoff 4 kHz @ 32 kHz,
Q=0.707).  Its poles have radius sqrt(a2) = 0.577, so the impulse response
decays below 1e-15 after 64 taps: to fp32 precision the filter is exactly a
64-tap FIR, y[t] = sum_j h[j] x[t-j].

Per 128-sample chunk c this is two banded-Toeplitz matmuls:
    y_c = W0^T x_c + W1^T x_{c-1}
with W0[pi,po] = h[po-pi] (lower band) and W1[pi,po] = h[128+po-pi]
(upper-right corner from the previous chunk).

On TRN2 we run this on the TensorEngine with the *data as the stationary
operand* and the W matrices as the moving operand:
    psum[c, fo] = sum_pi X_T[pi, c] * W[pi, fo]
so the output lands in PSUM already in natural [chunk, time] order - no
output transpose.  The input needs fine-time on partitions, which we get for
free from the DMA x-bar transpose (fp16).

Precision (LPF_TERMS):
  1: x and W in fp16, 2 matmuls/group              (~2e-4 rel err, fastest)
  3: x = xh+xl (fp16 pair), W = Wf + dW (fp16 pair), 6 matmuls/group,
     dropping only the dW*xl cross term            (~3e-7 rel err)

Batch dim (256 clips) is sharded 32 clips per NeuronCore, 8 cores SPMD.
"""

import os
import sys

sys.path.insert(0, "/opt/trn_rl_repo")

import numpy as np

import concourse.bacc as bacc
import concourse.mybir as mybir
import concourse.tile as tile
from concourse.bass_utils import run_bass_kernel_spmd

# ---- problem constants (hardcoded per contest rules) ----
B, T = 256, 160000
N_CORES = 8
CLIPS = B // N_CORES  # 32 clips per core
CH = 128  # chunk length (matmul contraction)
NCHUNK = T // CH  # 1250 data chunks per clip
# chunk slots per clip: 1 zero slot + 1250 data + 45 pad -> multiple of 16
# (DMA x-bar transpose needs source rows % 16 == 0)
S = 1296
NG = 10  # output groups of 128 chunks (cover slots 1..1280)
YCOLS = NG * CH  # 1280 output chunk slots per clip
TAPS = 64
CPB = 2  # clips per DMA batch
TERMS = int(os.environ.get("LPF_TERMS", "3"))

SAMPLE_RATE, CUTOFF, Q_VAL = 32000, 4000.0, 0.707


def _coeffs():
    # identical computation to the reference (incl. its float32 rounding)
    w0 = 2.0 * np.pi * CUTOFF / SAMPLE_RATE
    alpha = np.sin(w0) / (2.0 * Q_VAL)
    cosw = np.cos(w0)
    b0 = (1.0 - cosw) / 2.0
    b1 = 1.0 - cosw
    b2 = b0
    a0 = 1.0 + alpha
    a1 = -2.0 * cosw
    a2 = 1.0 - alpha
    return tuple(float(np.float32(c / a0)) for c in (b0, b1, b2, a1, a2))


def _impulse_response():
    b0, b1, b2, a1, a2 = _coeffs()
    h = np.zeros(TAPS, np.float64)
    x1 = x2 = y1 = y2 = 0.0
    for n in range(TAPS):
        x = 1.0 if n == 0 else 0.0
        y = b0 * x + b1 * x1 + b2 * x2 - a1 * y1 - a2 * y2
        h[n] = y
        x2, x1 = x1, x
        y2, y1 = y1, y
    return h


def _toeplitz():
    h = _impulse_response()
    pi = np.arange(128)[:, None]
    po = np.arange(128)[None, :]
    j0 = po - pi
    j1 = 128 + po - pi
    W0 = np.where((j0 >= 0) & (j0 < TAPS), h[np.clip(j0, 0, TAPS - 1)], 0.0)
    W1 = np.where((j1 >= 0) & (j1 < TAPS), h[np.clip(j1, 0, TAPS - 1)], 0.0)
    return W0, W1


def _weights(terms):
    W0, W1 = _toeplitz()
    W0f = W0.astype(np.float16)
    W1f = W1.astype(np.float16)
    if terms == 1:
        return np.concatenate([W0f, W1f], axis=1)  # [128, 256]
    dW0 = (W0 - W0f.astype(np.float64)).astype(np.float16)
    dW1 = (W1 - W1f.astype(np.float64)).astype(np.float16)
    return np.concatenate([W0f, dW0, W1f, dW1], axis=1)  # [128, 512]


_CACHE = {}


def _build(terms):
    dt = mybir.dt
    nc = bacc.Bacc(
        "TRN2", target_bir_lowering=False, debug=False, num_devices=N_CORES
    )
    # terms==3: xh and xl are packed per clip along the slot dim (2S slots per
    # clip).  The fine-time-on-partition transpose is done on the HOST: xh is
    # uploaded as [128 fine, CLIPS, SS slots] so loads are plain full-rate
    # DMAs with ~20KB contiguous runs per partition (no x-bar transpose).
    SS = S if terms == 1 else 2 * S
    xh = nc.dram_tensor("xh", [CH, CLIPS, SS], dt.float16, kind="ExternalInput").ap()
    wcols = 256 if terms == 1 else 512
    w = nc.dram_tensor("w", [128, wcols], dt.float16, kind="ExternalInput").ap()
    # y is written partition-major ([p, clip, g, f]) so each partition owns a
    # contiguous HBM span (big descriptors, full-rate); host de-scrambles.
    y = nc.dram_tensor(
        "y", [CH, CLIPS, NG, CH], dt.float32, kind="ExternalOutput"
    ).ap()

    NB = CLIPS // CPB
    with tile.TileContext(nc, trace_sim=False) as tc:
        with (
            tc.tile_pool(name="wpool", bufs=1) as wpool,
            tc.tile_pool(name="xpool", bufs=3) as xpool,
            tc.tile_pool(name="ypool", bufs=3) as ypool,
            tc.tile_pool(name="psum", bufs=6, space="PSUM") as ppool,
        ):
            wt = wpool.tile([128, wcols], dt.float16)
            nc.sync.dma_start(out=wt[:], in_=w)
            if terms == 1:
                W0f = wt[:, 0:128]
                W1f = wt[:, 128:256]
            else:
                W0f = wt[:, 0:128]
                dW0 = wt[:, 128:256]
                W1f = wt[:, 256:384]
                dW1 = wt[:, 384:512]

            for b in range(NB):
                src = xh[:, b * CPB : (b + 1) * CPB].rearrange("p c s -> p (c s)")
                xt = xpool.tile([128, CPB * SS], dt.float16, tag="xh")
                nc.sync.dma_start(out=xt[:], in_=src)
                xlt = xt  # xl cols live at +S within each clip's 2S span

                yt = ypool.tile([128, CPB * YCOLS], dt.float32)
                for c in range(CPB):
                    coff = c * SS + 1  # slot 0 of each clip is zeros
                    loff = S  # xl offset within the clip span
                    for pt_i in range(3):  # psum tiles: groups 0-3, 4-7, 8-9
                        g0 = pt_i * 4
                        ng = min(4, NG - g0)
                        pt = ppool.tile([128, 512], dt.float32)
                        for q in range(ng):
                            g = g0 + q
                            base = coff + g * 128
                            o = pt[:, q * 128 : (q + 1) * 128]
                            cur = (base, base + 128)
                            prv = (base - 1, base + 127)
                            if terms == 1:
                                nc.tensor.matmul(
                                    o, lhsT=xt[:, cur[0] : cur[1]], rhs=W0f,
                                    start=True, stop=False,
                                )
                                nc.tensor.matmul(
                                    o, lhsT=xt[:, prv[0] : prv[1]], rhs=W1f,
                                    start=False, stop=True,
                                )
                            else:
                                lcur = (cur[0] + loff, cur[1] + loff)
                                lprv = (prv[0] + loff, prv[1] + loff)
                                nc.tensor.matmul(
                                    o, lhsT=xt[:, cur[0] : cur[1]], rhs=W0f,
                                    start=True, stop=False,
                                )
                                nc.tensor.matmul(
                                    o, lhsT=xt[:, cur[0] : cur[1]], rhs=dW0,
                                    start=False, stop=False,
                                )
                                nc.tensor.matmul(
                                    o, lhsT=xlt[:, lcur[0] : lcur[1]], rhs=W0f,
                                    start=False, stop=False,
                                )
                                nc.tensor.matmul(
                                    o, lhsT=xt[:, prv[0] : prv[1]], rhs=W1f,
                                    start=False, stop=False,
                                )
                                nc.tensor.matmul(
                                    o, lhsT=xt[:, prv[0] : prv[1]], rhs=dW1,
                                    start=False, stop=False,
                                )
                                nc.tensor.matmul(
                                    o, lhsT=xlt[:, lprv[0] : lprv[1]], rhs=W1f,
                                    start=False, stop=True,
                                )
                        nc.any.tensor_copy(
                            yt[
                                :,
                                c * YCOLS + g0 * 128 : c * YCOLS + (g0 + ng) * 128,
                            ],
                            pt[:, : ng * 128],
                        )
                dst = y[:, b * CPB : (b + 1) * CPB].rearrange("p c g f -> p (c g f)")
                # scalar-engine HWDGE ring: runs concurrently with the input
                # loads on the sync-engine ring
                nc.scalar.dma_start(out=dst, in_=yt[:])
    nc.compile()
    return nc


def _get_nc(terms):
    if terms not in _CACHE:
        _CACHE[terms] = _build(terms)
    return _CACHE[terms]


def _marshal(clip, terms):
    clip = np.asarray(clip, dtype=np.float32)
    assert clip.shape == (B, T)
    xp = np.zeros((B, S, CH), np.float32)
    xp[:, 1 : 1 + NCHUNK, :] = clip.reshape(B, NCHUNK, CH)
    xh = xp.astype(np.float16)
    wcat = _weights(terms)
    in_maps = []
    if terms == 3:
        xlf = (xp - xh.astype(np.float32)).astype(np.float16)
        xcat = np.concatenate([xh, xlf], axis=1)  # [B, 2S, CH]
        xT = np.ascontiguousarray(xcat.transpose(2, 0, 1))  # [CH, B, 2S]
        for k in range(N_CORES):
            sl = slice(k * CLIPS, (k + 1) * CLIPS)
            in_maps.append({"xh": np.ascontiguousarray(xT[:, sl]), "w": wcat})
    else:
        xT = np.ascontiguousarray(xh.transpose(2, 0, 1))  # [CH, B, S]
        for k in range(N_CORES):
            sl = slice(k * CLIPS, (k + 1) * CLIPS)
            in_maps.append({"xh": np.ascontiguousarray(xT[:, sl]), "w": wcat})
    return in_maps


def _gather(results):
    outs = []
    for k in range(N_CORES):
        yk = np.asarray(results[k]["y"], dtype=np.float32)
        # [128p, CLIPS, NG, 128f] -> [c, g, p, f]; chunk slot = g*128+p
        yk = yk.reshape(CH, CLIPS, NG, CH).transpose(1, 2, 0, 3)
        yk = yk.reshape(CLIPS, YCOLS, CH)[:, :NCHUNK, :].reshape(CLIPS, T)
        outs.append(yk)
    return np.concatenate(outs, axis=0)


def run(clip, terms=None, trace=False):
    terms = TERMS if terms is None else terms
    nc = _get_nc(terms)
    in_maps = _marshal(clip, terms)
    res = run_bass_kernel_spmd(
        nc, in_maps, list(range(N_CORES)), trace=trace
    )
    return _gather(res.results), res.exec_time_ns


def kernel(clip):
    out, _ = run(clip)
    return out


# revision 16
# speedup vs baseline: 1.0737x; 1.0737x over previous
"""Trainium2 Bass kernel for a biquad lowpass filter over [256, 160000] audio.

Algorithm
---------
The reference is a Direct-Form-I biquad IIR (lowpass, cutoff 4 kHz @ 32 kHz,
Q=0.707).  Its poles have radius sqrt(a2) = 0.577, so the impulse response
decays below 1e-15 after 64 taps: to fp32 precision the filter is exactly a
64-tap FIR, y[t] = sum_j h[j] x[t-j].

Per 128-sample chunk c this is two banded-Toeplitz matmuls:
    y_c = W0^T x_c + W1^T x_{c-1}
with W0[pi,po] = h[po-pi] (lower band) and W1[pi,po] = h[128+po-pi]
(upper-right corner from the previous chunk).

On TRN2 we run this on the TensorEngine with the *data as the stationary
operand* and the W matrices as the moving operand:
    psum[c, fo] = sum_pi X_T[pi, c] * W[pi, fo]
so the output lands in PSUM already in natural [chunk, time] order - no
output transpose.  The input needs fine-time on partitions, which we get for
free from the DMA x-bar transpose (fp16).

Precision (LPF_TERMS):
  1: x and W in fp16, 2 matmuls/group              (~2e-4 rel err, fastest)
  3: x = xh+xl (fp16 pair), W = Wf + dW (fp16 pair), 6 matmuls/group,
     dropping only the dW*xl cross term            (~3e-7 rel err)

Batch dim (256 clips) is sharded 32 clips per NeuronCore, 8 cores SPMD.
"""

import os
import sys

sys.path.insert(0, "/opt/trn_rl_repo")

import numpy as np

import concourse.bacc as bacc
import concourse.mybir as mybir
import concourse.tile as tile
from concourse.bass_utils import run_bass_kernel_spmd

# ---- problem constants (hardcoded per contest rules) ----
B, T = 256, 160000
N_CORES = 8
CLIPS = B // N_CORES  # 32 clips per core
CH = 128  # chunk length (matmul contraction)
NCHUNK = T // CH  # 1250 data chunks per clip
# chunk slots per clip: 1 zero slot + 1250 data + 45 pad -> multiple of 16
# (DMA x-bar transpose needs source rows % 16 == 0)
S = 1296
NG = 10  # output groups of 128 chunks (cover slots 1..1280)
YCOLS = NG * CH  # 1280 output chunk slots per clip
TAPS = 64
CPB = 4  # clips per DMA batch
TERMS = int(os.environ.get("LPF_TERMS", "3"))

SAMPLE_RATE, CUTOFF, Q_VAL = 32000, 4000.0, 0.707


def _coeffs():
    # identical computation to the reference (incl. its float32 rounding)
    w0 = 2.0 * np.pi * CUTOFF / SAMPLE_RATE
    alpha = np.sin(w0) / (2.0 * Q_VAL)
    cosw = np.cos(w0)
    b0 = (1.0 - cosw) / 2.0
    b1 = 1.0 - cosw
    b2 = b0
    a0 = 1.0 + alpha
    a1 = -2.0 * cosw
    a2 = 1.0 - alpha
    return tuple(float(np.float32(c / a0)) for c in (b0, b1, b2, a1, a2))


def _impulse_response():
    b0, b1, b2, a1, a2 = _coeffs()
    h = np.zeros(TAPS, np.float64)
    x1 = x2 = y1 = y2 = 0.0
    for n in range(TAPS):
        x = 1.0 if n == 0 else 0.0
        y = b0 * x + b1 * x1 + b2 * x2 - a1 * y1 - a2 * y2
        h[n] = y
        x2, x1 = x1, x
        y2, y1 = y1, y
    return h


def _toeplitz():
    h = _impulse_response()
    pi = np.arange(128)[:, None]
    po = np.arange(128)[None, :]
    j0 = po - pi
    j1 = 128 + po - pi
    W0 = np.where((j0 >= 0) & (j0 < TAPS), h[np.clip(j0, 0, TAPS - 1)], 0.0)
    W1 = np.where((j1 >= 0) & (j1 < TAPS), h[np.clip(j1, 0, TAPS - 1)], 0.0)
    return W0, W1


def _weights(terms):
    W0, W1 = _toeplitz()
    W0f = W0.astype(np.float16)
    W1f = W1.astype(np.float16)
    if terms == 1:
        return np.concatenate([W0f, W1f], axis=1)  # [128, 256]
    dW0 = (W0 - W0f.astype(np.float64)).astype(np.float16)
    dW1 = (W1 - W1f.astype(np.float64)).astype(np.float16)
    return np.concatenate([W0f, dW0, W1f, dW1], axis=1)  # [128, 512]


_CACHE = {}


def _build(terms):
    dt = mybir.dt
    nc = bacc.Bacc(
        "TRN2", target_bir_lowering=False, debug=False, num_devices=N_CORES
    )
    # terms==3: xh and xl are packed per clip along the slot dim (2S slots per
    # clip).  The fine-time-on-partition transpose is done on the HOST: xh is
    # uploaded as [128 fine, CLIPS, SS slots] so loads are plain full-rate
    # DMAs with ~20KB contiguous runs per partition (no x-bar transpose).
    SS = S if terms == 1 else 2 * S
    xh = nc.dram_tensor("xh", [CH, CLIPS, SS], dt.float16, kind="ExternalInput").ap()
    wcols = 256 if terms == 1 else 512
    w = nc.dram_tensor("w", [128, wcols], dt.float16, kind="ExternalInput").ap()
    # y is written partition-major ([p, clip, g, f]) so each partition owns a
    # contiguous HBM span (big descriptors, full-rate); host de-scrambles.
    y = nc.dram_tensor(
        "y", [CH, CLIPS, NG, CH], dt.float32, kind="ExternalOutput"
    ).ap()

    NB = CLIPS // CPB
    with tile.TileContext(nc, trace_sim=False) as tc:
        with (
            tc.tile_pool(name="wpool", bufs=1) as wpool,
            tc.tile_pool(name="xpool", bufs=4) as xpool,
            tc.tile_pool(name="ypool", bufs=3) as ypool,
            tc.tile_pool(name="psum", bufs=6, space="PSUM") as ppool,
        ):
            wt = wpool.tile([128, wcols], dt.float16)
            nc.sync.dma_start(out=wt[:], in_=w)
            if terms == 1:
                W0f = wt[:, 0:128]
                W1f = wt[:, 128:256]
            else:
                W0f = wt[:, 0:128]
                dW0 = wt[:, 128:256]
                W1f = wt[:, 256:384]
                dW1 = wt[:, 384:512]

            for b in range(NB):
                src = xh[:, b * CPB : (b + 1) * CPB].rearrange("p c s -> p (c s)")
                xt = xpool.tile([128, CPB * SS], dt.float16, tag="xh")
                nc.sync.dma_start(out=xt[:], in_=src)
                xlt = xt  # xl cols live at +S within each clip's 2S span

                yt = ypool.tile([128, CPB * YCOLS], dt.float32)
                for c in range(CPB):
                    coff = c * SS + 1  # slot 0 of each clip is zeros
                    loff = S  # xl offset within the clip span
                    for pt_i in range(3):  # psum tiles: groups 0-3, 4-7, 8-9
                        g0 = pt_i * 4
                        ng = min(4, NG - g0)
                        pt = ppool.tile([128, 512], dt.float32)
                        for q in range(ng):
                            g = g0 + q
                            base = coff + g * 128
                            o = pt[:, q * 128 : (q + 1) * 128]
                            cur = (base, base + 128)
                            prv = (base - 1, base + 127)
                            if terms == 1:
                                nc.tensor.matmul(
                                    o, lhsT=xt[:, cur[0] : cur[1]], rhs=W0f,
                                    start=True, stop=False,
                                )
                                nc.tensor.matmul(
                                    o, lhsT=xt[:, prv[0] : prv[1]], rhs=W1f,
                                    start=False, stop=True,
                                )
                            else:
                                lcur = (cur[0] + loff, cur[1] + loff)
                                lprv = (prv[0] + loff, prv[1] + loff)
                                nc.tensor.matmul(
                                    o, lhsT=xt[:, cur[0] : cur[1]], rhs=W0f,
                                    start=True, stop=False,
                                )
                                nc.tensor.matmul(
                                    o, lhsT=xt[:, cur[0] : cur[1]], rhs=dW0,
                                    start=False, stop=False,
                                )
                                nc.tensor.matmul(
                                    o, lhsT=xlt[:, lcur[0] : lcur[1]], rhs=W0f,
                                    start=False, stop=False,
                                )
                                nc.tensor.matmul(
                                    o, lhsT=xt[:, prv[0] : prv[1]], rhs=W1f,
                                    start=False, stop=False,
                                )
                                nc.tensor.matmul(
                                    o, lhsT=xt[:, prv[0] : prv[1]], rhs=dW1,
                                    start=False, stop=False,
                                )
                                nc.tensor.matmul(
                                    o, lhsT=xlt[:, lprv[0] : lprv[1]], rhs=W1f,
                                    start=False, stop=True,
                                )
                        nc.any.tensor_copy(
                            yt[
                                :,
                                c * YCOLS + g0 * 128 : c * YCOLS + (g0 + ng) * 128,
                            ],
                            pt[:, : ng * 128],
                        )
                dst = y[:, b * CPB : (b + 1) * CPB].rearrange("p c g f -> p (c g f)")
                # scalar-engine HWDGE ring: runs concurrently with the input
                # loads on the sync-engine ring
                nc.scalar.dma_start(out=dst, in_=yt[:])
    nc.compile()
    return nc


def _get_nc(terms):
    if terms not in _CACHE:
        _CACHE[terms] = _build(terms)
    return _CACHE[terms]


def _marshal(clip, terms):
    clip = np.asarray(clip, dtype=np.float32)
    assert clip.shape == (B, T)
    xp = np.zeros((B, S, CH), np.float32)
    xp[:, 1 : 1 + NCHUNK, :] = clip.reshape(B, NCHUNK, CH)
    xh = xp.astype(np.float16)
    wcat = _weights(terms)
    in_maps = []
    if terms == 3:
        xlf = (xp - xh.astype(np.float32)).astype(np.float16)
        xcat = np.concatenate([xh, xlf], axis=1)  # [B, 2S, CH]
        xT = np.ascontiguousarray(xcat.transpose(2, 0, 1))  # [CH, B, 2S]
        for k in range(N_CORES):
            sl = slice(k * CLIPS, (k + 1) * CLIPS)
            in_maps.append({"xh": np.ascontiguousarray(xT[:, sl]), "w": wcat})
    else:
        xT = np.ascontiguousarray(xh.transpose(2, 0, 1))  # [CH, B, S]
        for k in range(N_CORES):
            sl = slice(k * CLIPS, (k + 1) * CLIPS)
            in_maps.append({"xh": np.ascontiguousarray(xT[:, sl]), "w": wcat})
    return in_maps


def _gather(results):
    outs = []
    for k in range(N_CORES):
        yk = np.asarray(results[k]["y"], dtype=np.float32)
        # [128p, CLIPS, NG, 128f] -> [c, g, p, f]; chunk slot = g*128+p
        yk = yk.reshape(CH, CLIPS, NG, CH).transpose(1, 2, 0, 3)
        yk = yk.reshape(CLIPS, YCOLS, CH)[:, :NCHUNK, :].reshape(CLIPS, T)
        outs.append(yk)
    return np.concatenate(outs, axis=0)


def run(clip, terms=None, trace=False):
    terms = TERMS if terms is None else terms
    nc = _get_nc(terms)
    in_maps = _marshal(clip, terms)
    res = run_bass_kernel_spmd(
        nc, in_maps, list(range(N_CORES)), trace=trace
    )
    return _gather(res.results), res.exec_time_ns


def kernel(clip):
    out, _ = run(clip)
    return out
